# revision 4
# baseline (speedup 1.0000x reference)
"""V5: bf16, fully SBUF-resident, software-pipelined across phases.

Key structure vs V4:
- Emission order interleaves phase 2 (attention) and phase 3 (output proj)
  chunks between phase-1 blocks so every engine stream stays dense:
    blk0 blk1 [A: attn(bh0,s0) attn(bh1,s0) proj(b0,s0)] blk2 [B: ...s1]
    blk3 [C] blk4 [D] blk5 [E: attn(bh2,s0)...] blk6 [F] blk7 [G] [H]
- Softmax denominator ones-matmul removed (-29us PE): exp tiles are
  accumulated on DVE (even k) + Pool (odd k) into a [P,2,QS] f32 acc,
  partition-summed by GPSIMD partition_all_reduce, merged on DVE.
- Causal diag masking is multiplicative 0/1 bf16 on et AFTER exp (single
  full-range exp per k-tile, no -30000 bias pass).
- Score->exp->AV chain software-pipelined (score k+1 emitted before AV k).
"""

import math
from contextlib import ExitStack

import numpy as np
import ml_dtypes

import concourse.bass as bass
import concourse.tile as tile
from concourse import bacc, mybir
from concourse.bass_isa import ReduceOp
from concourse.bass_utils import run_bass_kernel_spmd

B, L, H, NH, HD = 2, 2048, 2048, 16, 128
ROPE_THETA = 10000.0
N_CORES = 8
NH_LOC = NH // N_CORES          # 2
QKV_LOC = 3 * NH_LOC * HD       # 768
D_LOC = NH_LOC * HD             # 256
BL = B * L
P = 128
KC = H // P                     # 16
BLK = 512
NBLK = BL // BLK                # 8
BLK_PER_B = NBLK // B           # 4
QS = 512
NQS = L // QS
KT = L // P
NBH = B * NH_LOC                # 4

F32 = mybir.dt.float32
BF16 = mybir.dt.bfloat16
EXP = mybir.ActivationFunctionType.Exp
NP_BF16 = ml_dtypes.bfloat16


def _build():
    nc = bacc.Bacc("TRN2", target_bir_lowering=False, debug=False,
                   num_devices=N_CORES)

    xT = nc.dram_tensor("xT", [H, BL], BF16, kind="ExternalInput").ap()
    wT = nc.dram_tensor("wT", [H, QKV_LOC], BF16, kind="ExternalInput").ap()
    woT = nc.dram_tensor("woT", [D_LOC, H], BF16, kind="ExternalInput").ap()
    cosT = nc.dram_tensor("cosT", [HD, L], BF16, kind="ExternalInput").ap()
    sinTs = nc.dram_tensor("sinTs", [HD, L], BF16, kind="ExternalInput").ap()
    binm = nc.dram_tensor("binm", [P, P], BF16, kind="ExternalInput").ap()
    ones_in = nc.dram_tensor("ones", [P, P], BF16, kind="ExternalInput").ap()
    ones32_in = nc.dram_tensor("ones32", [P, P], mybir.dt.float32r,
                               kind="ExternalInput").ap()
    y = nc.dram_tensor("y", [BL, H], BF16, kind="ExternalOutput").ap()

    with tile.TileContext(nc) as tc, ExitStack() as ctx:
        g = ctx.enter_context(tc.tile_pool(name="g", bufs=1))
        kt_all = g.tile([P, NBH, L], BF16)        # rope'd K, hd-major
        qt_all = g.tile([P, NBH, L], BF16)        # rope'd Q, hd-major
        vt_all = g.tile([P, B, KT, D_LOC], BF16)  # V, token-major
        ot_all = g.tile([P, NBH, L], BF16)        # attn out, hd-major

        p2c = ctx.enter_context(tc.tile_pool(name="p2c", bufs=1))
        p2e = ctx.enter_context(tc.tile_pool(name="p2e", bufs=6))
        p2a = ctx.enter_context(tc.tile_pool(name="p2a", bufs=2))
        p3w = ctx.enter_context(tc.tile_pool(name="p3w", bufs=1))
        p3y = ctx.enter_context(tc.tile_pool(name="p3y", bufs=3))
        p1w = ctx.enter_context(tc.tile_pool(name="p1w", bufs=1))
        p1x = ctx.enter_context(tc.tile_pool(name="p1x", bufs=3))
        p1t = ctx.enter_context(tc.tile_pool(name="p1t", bufs=2))
        ps1 = ctx.enter_context(tc.tile_pool(name="ps1", bufs=2, space="PSUM"))
        ps2s = ctx.enter_context(tc.tile_pool(name="ps2s", bufs=2, space="PSUM"))
        ps2o = ctx.enter_context(tc.tile_pool(name="ps2o", bufs=1, space="PSUM"))
        ps2d = ctx.enter_context(tc.tile_pool(name="ps2d", bufs=1, space="PSUM"))
        ps3 = ctx.enter_context(tc.tile_pool(name="ps3", bufs=2, space="PSUM"))

        # prologue loads, ordered to feed blk0's matmul chains ASAP:
        # wt q/k cols in kc chunks interleaved with xb0 halves, then wt v
        # cols (needed ~14us in), cos/sin (DVE rope, off PE path), then
        # wo/mask (not needed until first attention/proj chunks).
        mask = p2c.tile([P, P], BF16)
        ones = p2c.tile([P, P], BF16)
        ones32 = p2c.tile([P, P], mybir.dt.float32r)
        wo = p3w.tile([P, NH_LOC, H], BF16)
        wt = p1w.tile([P, KC, QKV_LOC], BF16)
        cost = p1w.tile([P, L], BF16)
        sints = p1w.tile([P, L], BF16)
        xb_t = [None] * NBLK
        xb_t[0] = p1x.tile([P, KC, BLK], BF16, name="xb")

        def load_wt(k0, k1, c0, c1):
            nc.sync.dma_start(
                wt[:, k0:k1, c0:c1],
                wT[k0 * P:k1 * P, c0:c1].rearrange("(n p) d -> p n d", p=P))

        def load_xb(blk, half):
            k0, k1 = (0, KC // 2) if half == 0 else (KC // 2, KC)
            nc.sync.dma_start(
                xb_t[blk][:, k0:k1, :],
                xT[k0 * P:k1 * P, blk * BLK:(blk + 1) * BLK]
                .rearrange("(n p) t -> p n t", p=P))

        load_wt(0, 4, 0, 2 * D_LOC)
        load_xb(0, 0)
        load_wt(4, 10, 0, 2 * D_LOC)
        load_xb(0, 1)
        load_wt(10, KC, 0, 2 * D_LOC)
        load_wt(0, KC, 2 * D_LOC, 3 * D_LOC)
        nc.sync.dma_start(cost[:], cosT[:])
        nc.sync.dma_start(sints[:], sinTs[:])

        # ---------------- phase 1: one 512-token block ----------------
        def blk_body(blk):
            b, lo = divmod(blk, BLK_PER_B)
            lo *= BLK
            col = blk * BLK
            if xb_t[blk] is None:
                xb_t[blk] = p1x.tile([P, KC, BLK], BF16, name="xb")
                load_xb(blk, 0)
                load_xb(blk, 1)
            xb = xb_t[blk]

            qc = p1t.tile([P, 4, BLK], BF16, name="qc")
            for dt_i in range(4):       # 0,1 = q heads; 2,3 = k heads
                psum = ps1.tile([P, BLK], F32, name="ps")
                for kc in range(KC):
                    nc.tensor.matmul(
                        psum[:], lhsT=wt[:, kc, dt_i * P:dt_i * P + P],
                        rhs=xb[:, kc, :],
                        start=(kc == 0), stop=(kc == KC - 1))
                nc.scalar.copy(qc[:, dt_i, :], psum[:])
            qsw = p1t.tile([P, 4, BLK], BF16, name="qsw")
            nc.scalar.dma_start(qsw[0:64, :, :], qc[64:128, :, :])
            nc.scalar.dma_start(qsw[64:128, :, :], qc[0:64, :, :])

            for dt_i in range(4):
                qk, hh = divmod(dt_i, 2)
                bh = b * NH_LOC + hh
                t1 = p1t.tile([P, BLK], BF16, name="t1")
                nc.vector.tensor_mul(t1[:], qc[:, dt_i, :],
                                     cost[:, lo:lo + BLK])
                t2 = p1t.tile([P, BLK], BF16, name="t2")
                nc.vector.tensor_mul(t2[:], qsw[:, dt_i, :],
                                     sints[:, lo:lo + BLK])
                dst = qt_all if qk == 0 else kt_all
                nc.vector.tensor_add(dst[:, bh, lo:lo + BLK], t1[:], t2[:])

            for tt in range(BLK // P):
                psv = ps1.tile([P, BLK], F32, name="ps")
                for kc in range(KC):
                    nc.tensor.matmul(
                        psv[:, 0:D_LOC],
                        lhsT=xb[:, kc, tt * P:(tt + 1) * P],
                        rhs=wt[:, kc, 2 * D_LOC:3 * D_LOC],
                        start=(kc == 0), stop=(kc == KC - 1))
                if blk >= 6:
                    nc.vector.tensor_copy(vt_all[:, b, lo // P + tt, :],
                                          psv[:, 0:D_LOC])
                else:
                    nc.scalar.copy(vt_all[:, b, lo // P + tt, :],
                                   psv[:, 0:D_LOC])

        # ---------------- phase 2: attention for (bh, slice) ----------------
        def attn_body(bh, qs_i):
            b, hh = divmod(bh, NH_LOC)
            qs = qs_i * QS
            nkt = (qs + QS) // P
            kd0 = nkt - 4                   # first diagonal k-tile
            po = ps2o.tile([P, QS], F32, name="po")
            pd = ps2d.tile([P, QS], F32, name="pd")
            # tail chunks keep the per-k ones-matmul (PE idles there); the
            # rest accumulate the denominator on DVE (hidden under phase-1
            # PE) and finalize with a single ones-matmul per slice.
            dve_den = not (bh >= NH_LOC and qs_i >= 2)
            acc = p2a.tile([P, QS], mybir.dt.float32r, name="acc") \
                if dve_den else None
            psc_t = [None] * nkt
            et_t = [None] * nkt

            def score(k_i):
                c0 = max(k_i * P - qs, 0)
                psc_t[k_i] = ps2s.tile([P, QS], F32, name="psc")
                nc.tensor.matmul(
                    psc_t[k_i][:, c0:QS],
                    lhsT=kt_all[:, bh, k_i * P:(k_i + 1) * P],
                    rhs=qt_all[:, bh, qs + c0:qs + QS],
                    start=True, stop=True)

            def ex_av(k_i):
                d = k_i * P - qs
                c0 = max(d, 0)
                et = p2e.tile([P, QS], BF16, name="et")
                et_t[k_i] = et
                nc.scalar.activation(et[:, c0:QS], psc_t[k_i][:, c0:QS], EXP)
                if k_i >= kd0:
                    nc.vector.tensor_mul(et[:, d:d + P], et[:, d:d + P],
                                         mask[:])
                nc.tensor.matmul(
                    po[:, c0:QS],
                    lhsT=vt_all[:, b, k_i, hh * HD:(hh + 1) * HD],
                    rhs=et[:, c0:QS], start=(k_i == 0),
                    stop=(k_i == nkt - 1))
                if dve_den:
                    if k_i == 0:
                        nc.vector.tensor_copy(acc[:, c0:QS], et[:, c0:QS])
                    else:
                        nc.vector.tensor_add(acc[:, c0:QS], acc[:, c0:QS],
                                             et[:, c0:QS])
                else:
                    nc.tensor.matmul(pd[:, c0:QS], lhsT=ones[:],
                                     rhs=et[:, c0:QS], start=(k_i == 0),
                                     stop=(k_i == nkt - 1))

            score(0)
            for k_i in range(nkt):
                if k_i + 1 < nkt:
                    score(k_i + 1)
                ex_av(k_i)

            if dve_den:
                nc.tensor.matmul(pd[:], lhsT=ones32[:], rhs=acc[:],
                                 start=True, stop=True)
            rec = p2a.tile([P, QS], F32, name="rec")
            nc.vector.reciprocal(rec[:], pd[:])
            nc.vector.tensor_mul(ot_all[:, bh, qs:qs + QS], po[:], rec[:])

        # ---------------- phase 3: output proj for (b, slice) ----------------
        def proj_body(b, qs_i):
            for tt in range(qs_i * 4, qs_i * 4 + 4):
                ybig = p3y.tile([P, H], BF16, name="ybig")
                for oc in range(H // 512):
                    py_ = ps3.tile([P, 512], F32, name="py")
                    for hh in range(NH_LOC):
                        nc.tensor.matmul(
                            py_[:],
                            lhsT=ot_all[:, b * NH_LOC + hh,
                                        tt * P:(tt + 1) * P],
                            rhs=wo[:, hh, oc * 512:(oc + 1) * 512],
                            start=(hh == 0), stop=(hh == NH_LOC - 1))
                    sl = slice(oc * 512, (oc + 1) * 512)
                    if oc % 2 == 0:
                        nc.vector.tensor_copy(ybig[:, sl], py_[:])
                    else:
                        nc.scalar.copy(ybig[:, sl], py_[:])
                nc.scalar.dma_start(
                    y[b * L + tt * P: b * L + (tt + 1) * P, :], ybig[:])

        # ---------------- interleaved emission ----------------
        blk_body(0)
        blk_body(1)
        nc.sync.dma_start(mask[:], binm[:])
        nc.sync.dma_start(ones[:], ones_in[:])
        nc.sync.dma_start(ones32[:], ones32_in[:])
        for hh in range(NH_LOC):
            nc.sync.dma_start(wo[:, hh, :], woT[hh * P:(hh + 1) * P, :])
        # proj is staggered one slice behind attn so it never stalls on the
        # freshly-computed denominator chain; blk7 sits between the s2 and
        # s3 chunks of batch 1 so its PE work hides tail vector work.
        for s in range(NQS):            # batch 0 chunks A-D
            attn_body(0, s)
            attn_body(1, s)
            if s > 0:
                proj_body(0, s - 1)
            blk_body(s + 2)
        attn_body(2, 0)
        attn_body(3, 0)
        proj_body(0, 3)
        blk_body(6)
        attn_body(2, 1)
        attn_body(3, 1)
        proj_body(1, 0)
        attn_body(2, 2)
        attn_body(3, 2)
        proj_body(1, 1)
        blk_body(7)
        attn_body(2, 3)
        attn_body(3, 3)
        proj_body(1, 2)
        proj_body(1, 3)

    nc.compile()
    return nc


_NC = None


def _get_nc():
    global _NC
    if _NC is None:
        _NC = _build()
    return _NC


def _host_inputs(x, Wqkv, Wo):
    x = np.asarray(x, dtype=np.float32)
    Wqkv = np.asarray(Wqkv, dtype=np.float32)
    Wo = np.asarray(Wo, dtype=np.float32)

    xT = np.ascontiguousarray(x.reshape(BL, H).T).astype(NP_BF16)

    inv_freq = 1.0 / (ROPE_THETA ** (np.arange(0, HD, 2, dtype=np.float32)
                                     / HD))
    t = np.arange(L, dtype=np.float32)
    freqs = np.outer(t, inv_freq).astype(np.float32)
    emb = np.concatenate([freqs, freqs], axis=-1)
    cosT = np.ascontiguousarray(np.cos(emb).T).astype(NP_BF16)
    sinT = np.sin(emb).T.astype(np.float32)
    sinTs = np.ascontiguousarray(
        np.concatenate([-sinT[:64], sinT[64:]], 0)).astype(NP_BF16)

    kk = np.arange(P)[:, None]
    qq = np.arange(P)[None, :]
    binm = (qq >= kk).astype(NP_BF16)

    scale = np.float32(1.0 / math.sqrt(HD))
    in_maps = []
    for c in range(N_CORES):
        r0 = c * D_LOC
        wq = Wqkv[r0:r0 + D_LOC] * scale
        wk = Wqkv[H + r0:H + r0 + D_LOC]
        wv = Wqkv[2 * H + r0:2 * H + r0 + D_LOC]
        wT_c = np.ascontiguousarray(
            np.concatenate([wq, wk, wv], 0).T).astype(NP_BF16)
        woT_c = np.ascontiguousarray(Wo[:, r0:r0 + D_LOC].T).astype(NP_BF16)
        in_maps.append({
            "xT": xT, "wT": wT_c, "woT": woT_c,
            "cosT": cosT, "sinTs": sinTs, "binm": binm,
            "ones": np.ones((P, P), dtype=NP_BF16),
            "ones32": np.ones((P, P), dtype=np.float32),
        })
    return in_maps


def kernel(x, Wqkv, Wo):
    nc = _get_nc()
    in_maps = _host_inputs(x, Wqkv, Wo)
    res = run_bass_kernel_spmd(nc, in_maps, list(range(N_CORES)))
    y = res.results[0]["y"].astype(np.float64)
    for c in range(1, N_CORES):
        y += res.results[c]["y"]
    return y.astype(np.float32).reshape(B, L, H)
